# revision 1
# baseline (speedup 1.0000x reference)
"""Cross-covariance attention (XCA) kernel for Trainium2, 8 NeuronCores.

Problem (per batch element b, one per core — data-parallel over B=8):
    qkv = x @ Wqkv;  q,k,v heads of dim 64;  q,k L2-normalized over the
    TOKEN axis;  attn_h = softmax((k_h^T q_h) * temp_h) (64x64, head-local);
    y = concat_h(v_h @ attn_h) @ Wout + bout.

Algebraic reduction: the attention matrix only depends on the token
covariance C = x^T x (768x768):
    k_h^T q_h = Wk_h^T C Wq_h,   ||q_col_j||^2 = diag(Wq^T C Wq)_j
and the output collapses to y = x @ W3 + bout with
    W3 = Wv @ blockdiag(A_h) @ Wout.

v2 design vs baseline:
- Single pass over x: each token tile is loaded once (fp32->fp16 DMA
  cast), feeds the C accumulation, and is transposed into a persistent
  x^T SBUF tensor via the DMA XBAR transpose engine (InstDmaTransposeAnt,
  14ns per 16x128 tile) instead of PE transposes.  This removes the
  second 12.6MB HBM read of x and ~25us of PE transpose+LDWEIGHTS work.
- All other transposes (C mirror blocks, Wv^T, A^T) also go through the
  DMA XBAR on otherwise-idle HWDGE queues.
- Deferred normalization: the logits are computed from UNSCALED Wk/Mq
  (so the G matmuls overlap the norm-reduction), then scaled in G-space:
  columns by 1/||q|| (replicated vector), rows by temp/||k|| folded into
  the exp() activation's per-partition scale operand.  Softmax skips the
  max-subtraction (|logits| <= temp, exp cannot overflow).
- fp16 everywhere on the heavy paths (PE 1 cyc/row, 10 mantissa bits;
  every tensor here is O(10^3) max).  PSUM accumulation is fp32.
- Matmul loop orders keep the stationary operand constant across
  consecutive instructions where possible, and the walrus LDWEIGHTS
  dedup pass is enabled (--enable-ldw-opt=true) to skip redundant
  weight reloads.
"""

import os
import sys

sys.path.insert(0, "/opt/trn_rl_repo")

import numpy as np

import concourse.bacc as bacc
import concourse.bass as bass
import concourse.mybir as mybir
import concourse.tile as tile
from concourse.bass_utils import run_bass_kernel_spmd
from concourse.masks import make_identity

F32 = mybir.dt.float32
FP16 = mybir.dt.float16

B, N, D = 8, 4096, 768
H, DH = 12, 64
P = 128
KT = D // P  # 6 feature tiles
TT = N // P  # 32 token tiles
GRP = 4  # token tiles per load/transpose group
GT = TT // GRP  # 8 groups
HP = H // 2  # 6 head pairs (2 heads packed into 128 partitions)
EPS = 1e-12


if os.environ.get("BASS_LDW_OPT", "0") == "1":
    # Allow walrus to dedup back-to-back LDWEIGHTS with identical sources
    # (bass passes --enable-ldw-opt=false by default). Loop orders below are
    # arranged so consecutive matmuls share their stationary operand.
    import concourse.bass_utils as _bu

    if not getattr(_bu, "_ldw_opt_patched", False):
        _orig_run_command = _bu.run_command

        def _run_command_ldw(argv, **kwargs):
            argv = [
                a
                for a in argv
            ]
            return _orig_run_command(argv, **kwargs)

        _bu.run_command = _run_command_ldw
        _bu._ldw_opt_patched = True


def build_nc():
    nc = bacc.Bacc("TRN2", target_bir_lowering=False, debug=False)

    x_d = nc.dram_tensor("x", (N, D), F32, kind="ExternalInput")
    wqkv_d = nc.dram_tensor("wqkv", (D, 3 * D), F32, kind="ExternalInput")
    temp_d = nc.dram_tensor("temp", (H,), F32, kind="ExternalInput")
    wout_d = nc.dram_tensor("wout", (D, D), F32, kind="ExternalInput")
    bout_d = nc.dram_tensor("bout", (D,), F32, kind="ExternalInput")
    y_d = nc.dram_tensor("y", (N, D), F32, kind="ExternalOutput")

    with tile.TileContext(nc) as tc:
        _emit(tc, nc, x_d, wqkv_d, temp_d, wout_d, bout_d, y_d)
    nc.compile()
    return nc


def _emit(tc, nc, x_d, wqkv_d, temp_d, wout_d, bout_d, y_d):
    from contextlib import ExitStack

    ctx = ExitStack()
    with ctx:
        # ---------------- pools ----------------
        persist = ctx.enter_context(tc.tile_pool(name="persist", bufs=1))
        small = ctx.enter_context(tc.tile_pool(name="small", bufs=1))
        xgpool = ctx.enter_context(tc.tile_pool(name="xgpool", bufs=6))
        tmppool = ctx.enter_context(tc.tile_pool(name="tmppool", bufs=2))
        sfpool = ctx.enter_context(tc.tile_pool(name="sfpool", bufs=3))
        ypool = ctx.enter_context(tc.tile_pool(name="ypool", bufs=3))

        xtt = persist.tile([P, TT, KT, P], FP16)  # x^T, tile-major
        wqk_sb = persist.tile([P, KT, 2 * D], FP16)  # [Wq | Wk]
        c_sb = persist.tile([P, KT, D], FP16)  # C = x^T x
        mq_sb = persist.tile([P, KT, D], FP16)  # Mq = C @ Wq
        mk_sb = persist.tile([P, KT, D], FP16)  # Mk = C @ Wk
        wv_sb = persist.tile([P, KT, D], FP16)  # Wv (natural)
        wvt_sb = persist.tile([P, KT, D], FP16)  # Wv^T
        wout_sb = persist.tile([P, KT, D], FP16)  # Wout (natural)
        w2_sb = persist.tile([P, KT, D], FP16)  # blockdiag(A) @ Wout
        w3_sb = persist.tile([P, KT, D], FP16)  # W3 = Wv @ W2

        # small constants first so they never sit mid-stream in the
        # gpsimd queue
        ident32 = small.tile([P, P], F32)
        make_identity(nc, ident32)
        ident16 = small.tile([P, P], FP16)
        nc.vector.tensor_copy(ident16, ident32)
        ones16 = small.tile([P, P], FP16)
        nc.vector.memset(ones16, 1.0)
        temp_sb = small.tile([P, H], F32)
        nc.gpsimd.dma_start(temp_sb, temp_d[None, :].to_broadcast((P, H)))
        bout_sb = small.tile([P, D], F32)
        nc.gpsimd.dma_start(bout_sb, bout_d[None, :].to_broadcast((P, D)))

        # one cast-DMA per 4-tile group (a per-tile DMA costs ~0.7us of
        # SWDGE queue issue time; merged groups cut the queue load 4x)
        def load_group(g):
            xg = xgpool.tile([P, GRP, D], FP16, tag="xg", name="xg")
            nc.gpsimd.dma_start(
                xg,
                x_d[g * GRP * P : (g + 1) * GRP * P, :].rearrange(
                    "(tl p) c -> p tl c", p=P
                ),
            )
            return xg

        xg_head = [load_group(g) for g in range(4)]
        # s_sb: [1/max(nq,eps) | 1/max(nk,eps)], replicated on all partitions
        s_sb = small.tile([P, 2 * D], F32)
        eps2 = small.tile([P, 1], F32)
        nc.vector.memset(eps2, EPS * EPS)
        skd = small.tile([P, HP], F32)  # diag per head-pair: temp/nk at [p]
        # tdiag[p, hp] = temp[2*hp + p//64] (partition-indexed temperature)
        tdiag = small.tile([P, HP], F32)
        for hp in range(HP):
            nc.vector.tensor_copy(
                tdiag[0:64, hp : hp + 1], temp_sb[0:64, 2 * hp : 2 * hp + 1]
            )
            nc.vector.tensor_copy(
                tdiag[64:128, hp : hp + 1],
                temp_sb[64:128, 2 * hp + 1 : 2 * hp + 2],
            )

        # ------------- phase A: C = x^T x, upper block-triangle -------------
        # plus XBAR transposes of each loaded group into xtt (DMA engines)
        with tc.tile_pool(name="psC", bufs=1, space="PSUM") as psC:
            cps = [
                psC.tile([P, D - 128 * i], F32, name=f"cps{i}") for i in range(KT)
            ]
            xgs = list(xg_head)
            for g in range(GT):
                # issue loads 4 groups ahead (xgpool bufs=6 gives slack so
                # the gpsimd queue never blocks on buffer-reuse semaphores)
                if g + 4 < GT:
                    xgs.append(load_group(g + 4))
                # wqk is issued behind ~7 x-groups: it lands right before
                # Mqk needs it without delaying the x stream
                if g == 3:
                    nc.gpsimd.dma_start(
                        wqk_sb,
                        wqkv_d[:, 0 : 2 * D].rearrange("(ko p) c -> p ko c", p=P),
                    )
                # XBAR transpose this group into xtt; alternate between the
                # two HWDGE queues so neither serializes the stream
                xq = nc.sync if g % 2 == 0 else nc.scalar
                xq.dma_start_transpose(
                    xtt[:, g * GRP : (g + 1) * GRP, :, :].rearrange(
                        "p tl k n -> p (tl k) n"
                    ),
                    xgs[g].rearrange("p tl c -> p (tl c)"),
                )
                for j in range(GRP):
                    t = GRP * g + j
                    xb = xgs[g][:, j, :]
                    for i in range(KT):
                        w = D - 128 * i
                        for lo in range(0, w, 512):
                            hi = min(lo + 512, w)
                            nc.tensor.matmul(
                                cps[i][:, lo:hi],
                                xb[:, i * P : (i + 1) * P],
                                xb[:, 128 * i + lo : 128 * i + hi],
                                start=(t == 0),
                                stop=(t == TT - 1),
                            )
            # wv / wout behind the x stream; they land well before W2/W3
            nc.gpsimd.dma_start(
                wv_sb,
                wqkv_d[:, 2 * D : 3 * D].rearrange("(ko p) c -> p ko c", p=P),
            )
            nc.gpsimd.dma_start(
                wout_sb, wout_d.rearrange("(ho p) c -> p ho c", p=P)
            )
            for i in range(KT):
                nc.vector.tensor_copy(c_sb[:, i, 128 * i : D], cps[i])

        # mirror the lower block-triangle of C on the PE (it is idle right
        # here waiting for c_sb, and each DMA_TRANSPOSE costs ~2us of queue
        # serialization): block (j,i) = block (i,j)^T
        with tc.tile_pool(name="psTP", bufs=3, space="PSUM") as psTP:
            for i in range(KT):
                for j in range(i + 1, KT):
                    tpm = psTP.tile([P, P], FP16, tag="tp", name="tpm")
                    nc.tensor.transpose(tpm, c_sb[:, i, j * P : (j + 1) * P], ident16)
                    nc.vector.tensor_copy(c_sb[:, j, i * P : (i + 1) * P], tpm)
        # Wv^T via XBAR (scalar queue; runs during Mqk, gates only W3)
        for fi in range(KT):
            nc.scalar.dma_start_transpose(
                wvt_sb[:, :, fi * P : (fi + 1) * P], wv_sb[:, fi, :]
            )

        # ------ phase C: Mqk = C @ [Wq|Wk] ------
        # Pure matmul loop: psMQ is 3 tiles x 2 bufs (6 banks) so each
        # stationary C block feeds all three 512-wide chunks back-to-back
        # and the PE never stalls on the previous f-tile's PSUM copies.
        # Norms are computed afterwards from the persistent Mq/Mk.
        with tc.tile_pool(name="psMQ", bufs=2, space="PSUM") as psMQ:
            for f in range(KT):
                pa = [
                    psMQ.tile([P, 512], F32, tag=f"pmq{i}", name=f"pmq{i}")
                    for i in range(3)
                ]
                for k in range(KT):
                    lhs = c_sb[:, k, f * P : (f + 1) * P]
                    for nch in range(3):
                        nc.tensor.matmul(
                            pa[nch],
                            lhs,
                            wqk_sb[:, k, nch * 512 : (nch + 1) * 512],
                            start=(k == 0),
                            stop=(k == KT - 1),
                        )
                nc.vector.tensor_copy(mq_sb[:, f, 0:512], pa[0])
                nc.vector.tensor_copy(mq_sb[:, f, 512:768], pa[1][:, 0:256])
                nc.vector.tensor_copy(mk_sb[:, f, 0:256], pa[1][:, 256:512])
                nc.vector.tensor_copy(mk_sb[:, f, 256:768], pa[2])

        # ------ norms ------
        with tc.tile_pool(name="psN", bufs=1, space="PSUM") as psN:
            nrm_ps = psN.tile([P, 2 * D], F32)  # [nq^2 | nk^2], replicated
            for f in range(KT):
                wt = tmppool.tile([P, 2 * D], FP16, tag="wt", name="wt")
                nc.vector.tensor_mul(wt[:, 0:D], wqk_sb[:, f, 0:D], mq_sb[:, f, :])
                nc.vector.tensor_mul(
                    wt[:, D : 2 * D], wqk_sb[:, f, D : 2 * D], mk_sb[:, f, :]
                )
                for lo in range(0, 2 * D, 512):
                    nc.tensor.matmul(
                        nrm_ps[:, lo : lo + 512],
                        ones16,
                        wt[:, lo : lo + 512],
                        start=(f == 0),
                        stop=(f == KT - 1),
                    )
            nc.vector.tensor_copy(s_sb, nrm_ps)

        # ------ logits G + softmax + W2 ------
        with tc.tile_pool(name="psG", bufs=1, space="PSUM") as psG:
            gtile = psG.tile([P, 3, 2, P], F32, name="gtile")
            gps = [gtile[:, i] for i in range(3)]
            # G_hp = sum_f Wk[f,hp]^T Mq[f,hp]; overlaps the s-chain on
            # ACT/DVE
            for hp in range(HP):
                for f in range(KT):
                    nc.tensor.matmul(
                        gps[hp // 2][:, hp % 2, :],
                        wqk_sb[:, f, D + hp * P : D + (hp + 1) * P],
                        mq_sb[:, f, hp * P : (hp + 1) * P],
                        start=(f == 0),
                        stop=(f == KT - 1),
                    )
            # k-side scale: extract the per-partition diagonal of nk^2 FIRST,
            # then sqrt/reciprocal on [P, HP] only (the full replicated
            # [P,1536] DVE reciprocal costs ~9us)
            dscr = small.tile([P, P], F32)
            for hp in range(HP):
                nc.vector.tensor_mul(
                    dscr, s_sb[:, D + hp * P : D + (hp + 1) * P], ident32
                )
                nc.vector.tensor_reduce(
                    skd[:, hp : hp + 1],
                    dscr,
                    axis=mybir.AxisListType.X,
                    op=mybir.AluOpType.add,
                )
            # skd = temp[head(p)] / max(sqrt(nk2_diag), eps)
            nc.scalar.activation(
                skd, skd, mybir.ActivationFunctionType.Sqrt, bias=eps2
            )
            nc.vector.reciprocal(skd, skd)
            nc.vector.tensor_mul(skd, skd, tdiag)
            # q-side: sqrt all 768 at once (ACT), reciprocal chunked per hp
            # below so it pipelines with the softmax
            sq = s_sb[:, 0:D]
            nc.scalar.activation(
                sq, sq, mybir.ActivationFunctionType.Sqrt, bias=eps2
            )

            # ---- softmax per head pair + W2 = blockdiag(A) @ Wout ----------
            with tc.tile_pool(name="psW2", bufs=2, space="PSUM") as psW2:
                for hp in range(HP):
                    g_view = gps[hp // 2][:, hp % 2, :]
                    # 1/||q|| for this head pair's 128 columns (replicated)
                    nc.vector.reciprocal(
                        s_sb[:, hp * P : (hp + 1) * P],
                        s_sb[:, hp * P : (hp + 1) * P],
                    )
                    u = sfpool.tile([P, P], F32, tag="u", name="u")
                    # column scale by 1/||q|| (replicated vector)
                    nc.vector.tensor_mul(u, g_view, s_sb[:, hp * P : (hp + 1) * P])
                    # row scale by temp/||k|| (per-partition scalar)
                    nc.vector.tensor_scalar_mul(u, u, skd[:, hp : hp + 1])
                    a_bd = sfpool.tile([P, P], FP16, tag="a_bd", name="a_bd")
                    nc.vector.memset(a_bd, 0.0)
                    e_tmp = sfpool.tile([P, P], F32, tag="e", name="e_tmp")
                    for half in range(2):
                        lo64 = half * 64
                        u_blk = u[lo64 : lo64 + 64, lo64 : lo64 + 64]
                        e_blk = e_tmp[lo64 : lo64 + 64, lo64 : lo64 + 64]
                        sumexp = small.tile(
                            [P, 1], F32, tag="sumexp", name="sumexp", bufs=4
                        )
                        rec = small.tile([P, 1], F32, tag="rec", name="rec", bufs=4)
                        se = sumexp[lo64 : lo64 + 64]
                        rc = rec[lo64 : lo64 + 64]
                        # |logit| <= temp so no max-subtraction is needed
                        nc.scalar.activation(
                            e_blk,
                            u_blk,
                            mybir.ActivationFunctionType.Exp,
                            accum_out=se,
                        )
                        nc.vector.reciprocal(rc, se)
                        nc.vector.tensor_scalar_mul(
                            a_bd[lo64 : lo64 + 64, lo64 : lo64 + 64], e_blk, rc
                        )
                    at_bd = sfpool.tile([P, P], FP16, tag="at", name="at_bd")
                    atp = psW2.tile([P, P], FP16, tag="atp", name="atp")
                    nc.tensor.transpose(atp, a_bd, ident16)
                    nc.vector.tensor_copy(at_bd, atp)
                    w2ps = psW2.tile([P, D], F32, tag="w2ps", name="w2ps")
                    for lo, hi in ((0, 512), (512, 768)):
                        nc.tensor.matmul(
                            w2ps[:, lo:hi],
                            at_bd,
                            wout_sb[:, hp, lo:hi],
                            start=True,
                            stop=True,
                        )
                    nc.vector.tensor_copy(w2_sb[:, hp, :], w2ps)

        # ---------------- W3 = Wv @ W2 ----------------
        with tc.tile_pool(name="psW3", bufs=2, space="PSUM") as psW3:
            for fi in range(KT):
                w3ps = psW3.tile([P, D], F32, tag="w3ps", name="w3ps")
                for g in range(KT):
                    lhs = wvt_sb[:, g, fi * P : (fi + 1) * P]
                    for lo, hi in ((0, 512), (512, 768)):
                        nc.tensor.matmul(
                            w3ps[:, lo:hi],
                            lhs,
                            w2_sb[:, g, lo:hi],
                            start=(g == 0),
                            stop=(g == KT - 1),
                        )
                nc.vector.tensor_copy(w3_sb[:, fi, :], w3ps)

        # ---------------- phase E: y = x @ W3 + bout ------------------------
        with tc.tile_pool(name="psY", bufs=3, space="PSUM") as psY:
            for t in range(TT):
                yps = psY.tile([P, D], F32, tag="yps", name="yps")
                for k in range(KT):
                    lhs = xtt[:, t, k, :]
                    for lo, hi in ((0, 512), (512, 768)):
                        nc.tensor.matmul(
                            yps[:, lo:hi],
                            lhs,
                            w3_sb[:, k, lo:hi],
                            start=(k == 0),
                            stop=(k == KT - 1),
                        )
                ysb = ypool.tile([P, D], F32, tag="ysb", name="ysb")
                nc.vector.tensor_add(ysb, yps, bout_sb)
                nc.sync.dma_start(y_d[t * P : (t + 1) * P, :], ysb)


_NC_CACHE = {}


def _get_nc():
    if "nc" not in _NC_CACHE:
        _NC_CACHE["nc"] = build_nc()
    return _NC_CACHE["nc"]


def kernel_with_results(x, Wqkv, temperature, Wout, bout, **run_kwargs):
    x = np.ascontiguousarray(np.asarray(x, dtype=np.float32))
    Wqkv = np.ascontiguousarray(np.asarray(Wqkv, dtype=np.float32))
    temp = np.ascontiguousarray(np.asarray(temperature, dtype=np.float32).reshape(H))
    Wout = np.ascontiguousarray(np.asarray(Wout, dtype=np.float32))
    bout = np.ascontiguousarray(np.asarray(bout, dtype=np.float32))

    nc = _get_nc()
    in_maps = [
        {"x": x[b], "wqkv": Wqkv, "temp": temp, "wout": Wout, "bout": bout}
        for b in range(B)
    ]
    res = run_bass_kernel_spmd(nc, in_maps, core_ids=list(range(B)), **run_kwargs)
    out = np.stack([r["y"] for r in res.results], axis=0)
    return out, res


def kernel(x, Wqkv, temperature, Wout, bout):
    out, _ = kernel_with_results(x, Wqkv, temperature, Wout, bout)
    return out



# revision 2
# speedup vs baseline: 1.0350x; 1.0350x over previous
"""Cross-covariance attention (XCA) kernel for Trainium2, 8 NeuronCores.

Problem (per batch element b, one per core — data-parallel over B=8):
    qkv = x @ Wqkv;  q,k,v heads of dim 64;  q,k L2-normalized over the
    TOKEN axis;  attn_h = softmax((k_h^T q_h) * temp_h) (64x64, head-local);
    y = concat_h(v_h @ attn_h) @ Wout + bout.

Algebraic reduction: the attention matrix only depends on the token
covariance C = x^T x (768x768):
    k_h^T q_h = Wk_h^T C Wq_h,   ||q_col_j||^2 = diag(Wq^T C Wq)_j
and the output collapses to y = x @ W3 + bout with
    W3 = Wv @ blockdiag(A_h) @ Wout.

v3 design vs the 270us v2 baseline (trace-driven):
- x loads use consecutive-token-per-partition layout "(p tl) c" so each
  HBM descriptor is 12KB contiguous (vs 3KB) -> SDMA engines run at line
  rate and phase A is paced by C's PE work, not the loads.  The token
  permutation is free: C contracts over all tokens, and the y-tile
  writes invert it with strided row descriptors.
- Mqk = C @ [Wq|Wk] runs in fp8e4 with perf_mode=DoubleRow (256-row
  contraction per pass, ~2x).  C is scaled by 1/32 into fp8 (diag ~4096
  exceeds the 240 e4m3 max); the scale cancels exactly in the
  normalized logits G/(nq*nk).  Norm reductions for nk are interleaved
  into the Mqk stream (Mk is consumed from PSUM and never stored).
- The whole softmax is batched across all 6 head pairs: compact
  per-partition [P,6] norm diagonals (ident-mask + reduce), DVE
  reciprocals only on [P,6], the q-side 1/nq replicated via a
  diag(rq) @ ones identity-matmul broadcast, one [P,768] exp, masked
  block-diagonal A, 6 PE transposes, then W2 = A^T-blocks @ Wout.
- PE warmup burst at t=0 so the HAM clock-gate reaches 8/8 before the
  C accumulation starts; phases are arranged to keep PE gaps under the
  ~3.4us re-throttle window.
- SBUF pools are scoped: the x staging (49KB/partition) frees after
  phase A so everything fits in the 208KB/partition budget.
"""

import numpy as np

import concourse.bacc as bacc
import concourse.bass as bass
import concourse.mybir as mybir
import concourse.tile as tile
from concourse.bass_utils import run_bass_kernel_spmd
from concourse.masks import make_identity

F32 = mybir.dt.float32
FP16 = mybir.dt.float16
FP8 = mybir.dt.float8e4

B, N, D = 8, 4096, 768
H, DH = 12, 64
P = 128
KT = D // P  # 6 feature tiles
TT = N // P  # 32 token tiles
GRP = 4  # token tiles per load group
GT = TT // GRP  # 8 groups
HP = H // 2  # 6 head pairs (2 heads packed into 128 partitions)
EPS = 1e-12
CS = 1.0 / 32.0  # C -> fp8 scale (cancels in normalized logits)

MQK_FP8 = True  # DoubleRow fp8 for Mqk (fallback: fp16)


def build_nc():
    nc = bacc.Bacc("TRN2", target_bir_lowering=False, debug=False)

    x_d = nc.dram_tensor("x", (N, D), F32, kind="ExternalInput")
    wqkv_d = nc.dram_tensor("wqkv", (D, 3 * D), F32, kind="ExternalInput")
    temp_d = nc.dram_tensor("temp", (H,), F32, kind="ExternalInput")
    wout_d = nc.dram_tensor("wout", (D, D), F32, kind="ExternalInput")
    bout_d = nc.dram_tensor("bout", (D,), F32, kind="ExternalInput")
    y_d = nc.dram_tensor("y", (N, D), F32, kind="ExternalOutput")

    with tile.TileContext(nc) as tc:
        _emit(tc, nc, x_d, wqkv_d, temp_d, wout_d, bout_d, y_d)
    nc.compile()
    return nc


def _emit(tc, nc, x_d, wqkv_d, temp_d, wout_d, bout_d, y_d):
    from contextlib import ExitStack

    ctx = ExitStack()
    with ctx:
        # ---------------- persistent pools ----------------
        persist = ctx.enter_context(tc.tile_pool(name="persist", bufs=1))
        small = ctx.enter_context(tc.tile_pool(name="small", bufs=1))

        xtt = persist.tile([P, TT, KT, P], FP16)  # x^T, tile-major
        wqk_sb = persist.tile([P, KT, 2 * D], FP16)  # [Wq | Wk]
        c_sb = persist.tile([P, KT, D], FP16)  # C upper blocks (fp16)
        c8_sb = persist.tile([P, KT, D], FP8)  # C/32 full (fp8)
        wqk8 = persist.tile([P, KT, 2 * D], FP8)  # [Wq | Wk] fp8
        mq_sb = persist.tile([P, KT, D], FP16)  # Mq/32 = (C/32) @ Wq
        wvt_sb = persist.tile([P, KT, D], FP16)  # Wv^T
        wout_sb = persist.tile([P, KT, D], FP16)  # Wout (natural)
        w2_sb = persist.tile([P, KT, D], FP16)  # blockdiag(A) @ Wout
        w3_sb = persist.tile([P, KT, D], FP16)  # W3 = Wv @ W2

        # small constants first so they never sit mid-stream in the
        # gpsimd queue
        ident32 = small.tile([P, P], F32)
        make_identity(nc, ident32)
        ident16 = small.tile([P, P], FP16)
        nc.vector.tensor_copy(ident16, ident32)
        ones16 = small.tile([P, P], FP16)
        nc.vector.memset(ones16, 1.0)
        blockmask = small.tile([P, P], FP16)  # blockdiag(1_64, 1_64)
        nc.vector.memset(blockmask, 0.0)
        nc.vector.memset(blockmask[0:64, 0:64], 1.0)
        nc.vector.memset(blockmask[64:128, 64:128], 1.0)
        temp_sb = small.tile([P, H], F32)
        nc.gpsimd.dma_start(temp_sb, temp_d[None, :].to_broadcast((P, H)))
        bout_sb = small.tile([P, D], F32)
        nc.gpsimd.dma_start(bout_sb, bout_d[None, :].to_broadcast((P, D)))
        eps2 = small.tile([P, 1], F32)
        nc.vector.memset(eps2, EPS * EPS)
        # tdiag[p, hp] = temp[2*hp + p//64] (partition-indexed temperature)
        tdiag = small.tile([P, HP], F32)
        for hp in range(HP):
            nc.vector.tensor_copy(
                tdiag[0:64, hp : hp + 1], temp_sb[0:64, 2 * hp : 2 * hp + 1]
            )
            nc.vector.tensor_copy(
                tdiag[64:128, hp : hp + 1],
                temp_sb[64:128, 2 * hp + 1 : 2 * hp + 2],
            )

        # PE warmup: burn the HAM cold window on junk matmuls while the
        # first x group is still in flight, so C runs at 2.4GHz.
        with tc.tile_pool(name="psWarm", bufs=1, space="PSUM") as psWarm:
            wps = psWarm.tile([P, P], F32)
            for _ in range(30):
                nc.tensor.matmul(wps, ones16, ones16, start=True, stop=True)

        with tc.tile_pool(name="wvpool", bufs=1) as wvpool:
            wv_sb = wvpool.tile([P, KT, D], FP16)

            with tc.tile_pool(name="xgpool", bufs=GT) as xgpool:
                # one cast-DMA per group; "(p tl) c" gives each partition
                # GRP consecutive token rows = 12KB contiguous per
                # descriptor (token order inside the tile is permuted;
                # harmless for C, inverted at the y write).
                def load_group(g):
                    xg = xgpool.tile([P, GRP, D], FP16, tag="xg", name="xg")
                    nc.gpsimd.dma_start(
                        xg,
                        x_d[g * GRP * P : (g + 1) * GRP * P, :].rearrange(
                            "(p tl) c -> p tl c", p=P
                        ),
                    )
                    return xg

                xgs = [load_group(g) for g in range(GT)]
                # wqk behind the x stream, sliced so the fp8 casts can
                # chase the arrivals
                for s in range(KT):
                    nc.gpsimd.dma_start(
                        wqk_sb[:, s, :], wqkv_d[s * P : (s + 1) * P, 0 : 2 * D]
                    )
                # wv / wout behind wqk; needed only from W2/W3 onwards
                nc.gpsimd.dma_start(
                    wv_sb,
                    wqkv_d[:, 2 * D : 3 * D].rearrange("(ko p) c -> p ko c", p=P),
                )
                nc.gpsimd.dma_start(
                    wout_sb, wout_d.rearrange("(ho p) c -> p ho c", p=P)
                )
                # fp8 casts of wqk (DVE takes 3 slices, ACT the other 3)
                if MQK_FP8:
                    for s in range(KT):
                        if s % 2 == 0:
                            nc.vector.tensor_copy(wqk8[:, s, :], wqk_sb[:, s, :])
                        else:
                            nc.scalar.activation(
                                wqk8[:, s, :],
                                wqk_sb[:, s, :],
                                mybir.ActivationFunctionType.Copy,
                            )

                # ---- phase A: C = x^T x (upper block-triangle) ----
                # plus XBAR transposes of each group into xtt
                with tc.tile_pool(name="psC", bufs=1, space="PSUM") as psC:
                    cps = [
                        psC.tile([P, D - 128 * i], F32, name=f"cps{i}")
                        for i in range(KT)
                    ]
                    for g in range(GT):
                        xq = nc.sync if g % 2 == 0 else nc.scalar
                        xq.dma_start_transpose(
                            xtt[:, g * GRP : (g + 1) * GRP, :, :].rearrange(
                                "p tl k n -> p (tl k) n"
                            ),
                            xgs[g].rearrange("p tl c -> p (tl c)"),
                        )
                        for j in range(GRP):
                            t = GRP * g + j
                            xb = xgs[g][:, j, :]
                            for i in range(KT):
                                w = D - 128 * i
                                for lo in range(0, w, 512):
                                    hi = min(lo + 512, w)
                                    nc.tensor.matmul(
                                        cps[i][:, lo:hi],
                                        xb[:, i * P : (i + 1) * P],
                                        xb[:, 128 * i + lo : 128 * i + hi],
                                        start=(t == 0),
                                        stop=(t == TT - 1),
                                    )
                    for i in range(KT):
                        nc.vector.tensor_copy(c_sb[:, i, 128 * i : D], cps[i])
                        nc.vector.tensor_scalar_mul(
                            c8_sb[:, i, 128 * i : D], cps[i], CS
                        )

                # mirror the lower block-triangle on the PE (it is idle
                # waiting for the copies anyway): block (j,i) = (i,j)^T
                with tc.tile_pool(name="psTP", bufs=3, space="PSUM") as psTP:
                    for i in range(KT):
                        for j in range(i + 1, KT):
                            tpm = psTP.tile([P, P], FP16, tag="tp", name="tpm")
                            nc.tensor.transpose(
                                tpm, c_sb[:, i, j * P : (j + 1) * P], ident16
                            )
                            nc.vector.tensor_scalar_mul(
                                c8_sb[:, j, i * P : (i + 1) * P], tpm, CS
                            )
                            if not MQK_FP8:
                                nc.vector.tensor_copy(
                                    c_sb[:, j, i * P : (i + 1) * P], tpm
                                )
            # xgpool closed: 48KB/partition freed for the middle scratch

            with tc.tile_pool(name="midpool", bufs=1) as mid, tc.tile_pool(
                name="tmppool", bufs=2
            ) as tmppool:
                dscr = mid.tile([P, HP, P], F32)
                nq2c = mid.tile([P, HP], F32)
                nk2c = mid.tile([P, HP], F32)
                rq_c = mid.tile([P, HP], F32)
                skd = mid.tile([P, HP], F32)
                se_c = mid.tile([P, HP], F32)
                rse = mid.tile([P, HP], F32)
                dmat = mid.tile([P, HP, P], FP16)
                rq_sb = mid.tile([P, HP, P], FP16)
                u1 = mid.tile([P, HP, P], F32)
                e16 = mid.tile([P, HP, P], FP16)
                a_all = mid.tile([P, HP, P], FP16)
                at_sb = mid.tile([P, HP, P], FP16)

                # Wv^T via XBAR (scalar queue; gates only W3)
                for fi in range(KT):
                    nc.scalar.dma_start_transpose(
                        wvt_sb[:, :, fi * P : (fi + 1) * P], wv_sb[:, fi, :]
                    )

                # ---- Mqk = (C/32) @ [Wq|Wk], fp8 DoubleRow ----
                # nk norm reduction interleaved; Mk consumed from PSUM.
                with tc.tile_pool(
                    name="psMQ", bufs=2, space="PSUM"
                ) as psMQ, tc.tile_pool(name="psNK", bufs=1, space="PSUM") as psNK:
                    nrmk = psNK.tile([P, D], F32)
                    for f in range(KT):
                        pa = [
                            psMQ.tile([P, 512], F32, tag=f"pmq{i}", name=f"pmq{i}")
                            for i in range(3)
                        ]
                        if MQK_FP8:
                            for kp in range(3):
                                lhs = c8_sb[:, 2 * kp : 2 * kp + 2, f * P : (f + 1) * P]
                                for ch in range(3):
                                    nc.tensor.matmul(
                                        pa[ch],
                                        lhs,
                                        wqk8[:, 2 * kp : 2 * kp + 2, ch * 512 : (ch + 1) * 512],
                                        start=(kp == 0),
                                        stop=(kp == 2),
                                        perf_mode=mybir.MatmulPerfMode.DoubleRow,
                                    )
                        else:
                            for k in range(KT):
                                lhs = c_sb[:, k, f * P : (f + 1) * P]
                                for ch in range(3):
                                    nc.tensor.matmul(
                                        pa[ch],
                                        lhs,
                                        wqk_sb[:, k, ch * 512 : (ch + 1) * 512],
                                        start=(k == 0),
                                        stop=(k == KT - 1),
                                    )
                        nc.vector.tensor_copy(mq_sb[:, f, 0:512], pa[0])
                        nc.vector.tensor_copy(mq_sb[:, f, 512:768], pa[1][:, 0:256])
                        wtk = tmppool.tile([P, D], FP16, tag="wtk", name="wtk")
                        nc.vector.tensor_mul(
                            wtk[:, 0:256], wqk_sb[:, f, D : D + 256], pa[1][:, 256:512]
                        )
                        nc.vector.tensor_mul(
                            wtk[:, 256:768], wqk_sb[:, f, D + 256 : 2 * D], pa[2]
                        )
                        for lo, hi in ((0, 512), (512, 768)):
                            nc.tensor.matmul(
                                nrmk[:, lo:hi],
                                ones16,
                                wtk[:, lo:hi],
                                start=(f == 0),
                                stop=(f == KT - 1),
                            )
                    # nk^2 diagonal -> compact [P, HP] (while PE starts psG)
                    for hp in range(HP):
                        nc.vector.tensor_mul(
                            dscr[:, hp, :], nrmk[:, hp * P : (hp + 1) * P], ident32
                        )
                        nc.vector.tensor_reduce(
                            nk2c[:, hp : hp + 1],
                            dscr[:, hp, :],
                            axis=mybir.AxisListType.X,
                            op=mybir.AluOpType.add,
                        )

                # skd = temp[head(p)] / sqrt(nk2 + eps)
                nc.scalar.activation(
                    nk2c, nk2c, mybir.ActivationFunctionType.Sqrt, bias=eps2
                )
                nc.vector.reciprocal(skd, nk2c)
                nc.vector.tensor_mul(skd, skd, tdiag)

                # ---- nq norms + logits G + q-scale broadcast ----
                with tc.tile_pool(name="psG", bufs=1, space="PSUM") as psG:
                    nrmq = psG.tile([P, D], F32)
                    g_ps = psG.tile([P, HP, P], F32)
                    rq_ps = psG.tile([P, D], F32)
                    for f in range(KT):
                        wtq = tmppool.tile([P, D], FP16, tag="wtq", name="wtq")
                        nc.vector.tensor_mul(
                            wtq, wqk_sb[:, f, 0:D], mq_sb[:, f, :]
                        )
                        for lo, hi in ((0, 512), (512, 768)):
                            nc.tensor.matmul(
                                nrmq[:, lo:hi],
                                ones16,
                                wtq[:, lo:hi],
                                start=(f == 0),
                                stop=(f == KT - 1),
                            )
                    # G_hp = sum_f Wk[f,hp]^T Mq[f,hp]
                    for hp in range(HP):
                        for f in range(KT):
                            nc.tensor.matmul(
                                g_ps[:, hp, :],
                                wqk_sb[:, f, D + hp * P : D + (hp + 1) * P],
                                mq_sb[:, f, hp * P : (hp + 1) * P],
                                start=(f == 0),
                                stop=(f == KT - 1),
                            )
                    # nq^2 diagonal -> rq_c = 1/sqrt(nq2+eps), compact
                    for hp in range(HP):
                        nc.vector.tensor_mul(
                            dscr[:, hp, :], nrmq[:, hp * P : (hp + 1) * P], ident32
                        )
                        nc.vector.tensor_reduce(
                            nq2c[:, hp : hp + 1],
                            dscr[:, hp, :],
                            axis=mybir.AxisListType.X,
                            op=mybir.AluOpType.add,
                        )
                    nc.scalar.activation(
                        nq2c, nq2c, mybir.ActivationFunctionType.Sqrt, bias=eps2
                    )
                    nc.vector.reciprocal(rq_c, nq2c)
                    # replicate rq over partitions: ones^T @ diag(rq)
                    for hp in range(HP):
                        nc.vector.tensor_scalar_mul(
                            dmat[:, hp, :], ident32, rq_c[:, hp : hp + 1]
                        )
                    dm_flat = dmat.rearrange("p a b -> p (a b)")
                    for lo, hi in ((0, 512), (512, 768)):
                        nc.tensor.matmul(
                            rq_ps[:, lo:hi],
                            ones16,
                            dm_flat[:, lo:hi],
                            start=True,
                            stop=True,
                        )
                    nc.vector.tensor_copy(
                        rq_sb.rearrange("p a b -> p (a b)"), rq_ps
                    )
                    # u = G * (1/nq)[cols]
                    nc.vector.tensor_mul(u1, g_ps, rq_sb)

                # ---- batched softmax over all 6 head pairs ----
                for hp in range(HP):
                    nc.vector.tensor_scalar_mul(
                        u1[:, hp, :], u1[:, hp, :], skd[:, hp : hp + 1]
                    )
                # |logit| <= temp so exp cannot overflow; no max-subtract
                nc.scalar.activation(
                    e16, u1, mybir.ActivationFunctionType.Exp
                )
                for hp in range(HP):
                    nc.vector.tensor_mul(e16[:, hp, :], e16[:, hp, :], blockmask)
                    nc.vector.tensor_reduce(
                        se_c[:, hp : hp + 1],
                        e16[:, hp, :],
                        axis=mybir.AxisListType.X,
                        op=mybir.AluOpType.add,
                    )
                nc.vector.reciprocal(rse, se_c)
                for hp in range(HP):
                    nc.vector.tensor_scalar_mul(
                        a_all[:, hp, :], e16[:, hp, :], rse[:, hp : hp + 1]
                    )

                # ---- A^T + W2 = blockdiag(A) @ Wout ----
                with tc.tile_pool(name="psW2", bufs=2, space="PSUM") as psW2:
                    atp = psW2.tile([P, HP, P], FP16, name="atp")
                    for hp in range(HP):
                        nc.tensor.transpose(
                            atp[:, hp, :], a_all[:, hp, :], ident16
                        )
                    nc.vector.tensor_copy(
                        at_sb.rearrange("p a b -> p (a b)"),
                        atp.rearrange("p a b -> p (a b)"),
                    )
                    for hp in range(HP):
                        w2ps = psW2.tile([P, D], F32, tag="w2ps", name="w2ps")
                        for lo, hi in ((0, 512), (512, 768)):
                            nc.tensor.matmul(
                                w2ps[:, lo:hi],
                                at_sb[:, hp, :],
                                wout_sb[:, hp, lo:hi],
                                start=True,
                                stop=True,
                            )
                        nc.vector.tensor_copy(w2_sb[:, hp, :], w2ps)

                # ---------------- W3 = Wv @ W2 ----------------
                with tc.tile_pool(name="psW3", bufs=2, space="PSUM") as psW3:
                    for fi in range(KT):
                        w3ps = psW3.tile([P, D], F32, tag="w3ps", name="w3ps")
                        for g in range(KT):
                            lhs = wvt_sb[:, g, fi * P : (fi + 1) * P]
                            for lo, hi in ((0, 512), (512, 768)):
                                nc.tensor.matmul(
                                    w3ps[:, lo:hi],
                                    lhs,
                                    w2_sb[:, g, lo:hi],
                                    start=(g == 0),
                                    stop=(g == KT - 1),
                                )
                        nc.vector.tensor_copy(w3_sb[:, fi, :], w3ps)

        # ---------------- phase E: y = x @ W3 + bout --------------------
        with tc.tile_pool(name="ypool", bufs=3) as ypool, tc.tile_pool(
            name="psY", bufs=3, space="PSUM"
        ) as psY:
            for t in range(TT):
                g, tl = divmod(t, GRP)
                yps = psY.tile([P, D], F32, tag="yps", name="yps")
                for k in range(KT):
                    lhs = xtt[:, t, k, :]
                    for lo, hi in ((0, 512), (512, 768)):
                        nc.tensor.matmul(
                            yps[:, lo:hi],
                            lhs,
                            w3_sb[:, k, lo:hi],
                            start=(k == 0),
                            stop=(k == KT - 1),
                        )
                ysb = ypool.tile([P, D], F32, tag="ysb", name="ysb")
                nc.vector.tensor_add(ysb, yps, bout_sb)
                # invert the load permutation: partition p holds token
                # g*512 + 4p + tl
                yv = y_d[g * GRP * P : (g + 1) * GRP * P, :].rearrange(
                    "(p tl) c -> p tl c", p=P
                )[:, tl, :]
                nc.sync.dma_start(yv, ysb)


_NC_CACHE = {}


def _get_nc():
    if "nc" not in _NC_CACHE:
        _NC_CACHE["nc"] = build_nc()
    return _NC_CACHE["nc"]


def kernel_with_results(x, Wqkv, temperature, Wout, bout, **run_kwargs):
    x = np.ascontiguousarray(np.asarray(x, dtype=np.float32))
    Wqkv = np.ascontiguousarray(np.asarray(Wqkv, dtype=np.float32))
    temp = np.ascontiguousarray(np.asarray(temperature, dtype=np.float32).reshape(H))
    Wout = np.ascontiguousarray(np.asarray(Wout, dtype=np.float32))
    bout = np.ascontiguousarray(np.asarray(bout, dtype=np.float32))

    nc = _get_nc()
    in_maps = [
        {"x": x[b], "wqkv": Wqkv, "temp": temp, "wout": Wout, "bout": bout}
        for b in range(B)
    ]
    res = run_bass_kernel_spmd(nc, in_maps, core_ids=list(range(B)), **run_kwargs)
    out = np.stack([r["y"] for r in res.results], axis=0)
    return out, res


def kernel(x, Wqkv, temperature, Wout, bout):
    out, _ = kernel_with_results(x, Wqkv, temperature, Wout, bout)
    return out
